# revision 1
# baseline (speedup 1.0000x reference)
"""AttentiveManifoldMixer Trainium2 kernel (8-core data parallel over batch).

Math: with W3[c,i,j] = conv_w[c*64+i, j], B = conv_b.reshape(C, C),
  s[b]       = sigmoid(fc2 @ relu(fc1 @ mean_hw(x[b])))
  out[b,c,p] = sum_{i,j} W3[c,i,j] * s[b,j] * x[b,i,p] * x[b,j,p]
               + sum_i B[c,i] * x[b,i,p]

The quadratic form is symmetrized over unordered channel pairs grouped by
cyclic diagonal offset d: a feature lane holds x_i * x_j with j-i = d
(mod 64); the per-batch weight (W3[c,i,j]*s_j + W3[c,j,i]*s_i)/mult is
folded on device.  17 chunks x 128 lanes cover d = 0..33 (d=32/33 lanes are
duplicates at higher mult).  This halves the FLOPs of the naive C^2 conv.

Features are built from DMA-loaded rotated copies of a doubled bf16 x
([x; x], 128 rows) prestaged in DRAM.  Rotations are factored: chunk
m = 3k+l multiplies A_k = [rot_{-6k}(x)]x2 with B_l = [rot_2l(x);
rot_{2l+1}(x)] so 9 resident tiles serve all 17 chunks (one 128-lane bf16
tensor_tensor each).  GEMM: 17 bf16 matmuls (K=128, M=64, N=512) per PSUM
bank + a full-rate float32r conv_b matmul (identity init -> residual +x).
The whole dataflow (load/cast/prestage/variant-loads/features/GEMM/copy-out)
is pipelined in two 2048-pixel column halves so the first half's compute
overlaps the second half's DMA stream.
"""
import sys

sys.path.insert(0, "/opt/trn_rl_repo")

import numpy as np
import ml_dtypes

B, C, H, W = 8, 64, 64, 64
P = H * W                  # 4096 pixels per sample
MID = C // 4
NCHUNK = 17                # feature chunks
NA, NB = 6, 3              # A/B variant tiles; chunk m = 3*(m//3) + m%3
NSUB = 512                 # matmul free-dim subtile
NS = P // NSUB             # psum banks
N_CORES = 8

_CACHE = {}


def _lane_maps():
    """Per-lane (i, j, mult): chunk m = 3k+l, lane q = 64*qhi + qlo:
    i = (qlo - 6k) % 64,  j = (qlo + 2l + qhi) % 64."""
    i_idx = np.zeros((NCHUNK, 128), np.int64)
    j_idx = np.zeros((NCHUNK, 128), np.int64)
    for m in range(NCHUNK):
        k, l = divmod(m, 3)
        for q in range(128):
            qhi, qlo = divmod(q, 64)
            i_idx[m, q] = (qlo - 6 * k) % 64
            j_idx[m, q] = (qlo + 2 * l + qhi) % 64
    lo = np.minimum(i_idx, j_idx)
    hi = np.maximum(i_idx, j_idx)
    key = lo * 64 + hi
    _, inv, counts = np.unique(key, return_inverse=True, return_counts=True)
    mult = counts[inv].reshape(key.shape).astype(np.float32)
    return i_idx, j_idx, mult


def _host_weights(conv_w, fc1_w, fc2_w):
    """Pre-gather conv_w into per-lane arrays a1/a2 of shape (128, 17, 64):
    [lane q, chunk m, out-channel c], bf16."""
    w3 = conv_w.reshape(C, C, C)  # [c, i, j]
    i_idx, j_idx, mult = _lane_maps()
    a1 = np.transpose(w3[:, i_idx, j_idx], (2, 1, 0)) / mult.T[:, :, None]
    a2 = np.transpose(w3[:, j_idx, i_idx], (2, 1, 0)) / mult.T[:, :, None]
    diag = (i_idx == j_idx).T  # [q, m]
    a2[diag] = 0.0
    fc1t = (fc1_w.T / float(P)).copy()   # (64, 16): folds the 1/HW of the mean
    fc2t = fc2_w.T.copy()                # (16, 64)
    return (np.ascontiguousarray(a1, ml_dtypes.bfloat16),
            np.ascontiguousarray(a2, ml_dtypes.bfloat16), fc1t, fc2t)


def _build_program(niter=None):
    """Build the kernel program; with niter, wrap the body in an on-device
    For_i repeat loop (timing variant)."""
    import contextlib

    import concourse.bacc as bacc
    import concourse.bass as bass
    from concourse import mybir
    from concourse.tile import TileContext

    nc = bacc.Bacc("TRN2", target_bir_lowering=False, debug=False)
    dt = mybir.dt

    x_d = nc.dram_tensor("x", [C, P], dt.float32r, kind="ExternalInput")
    a1_d = nc.dram_tensor("a1", [128, NCHUNK, C], dt.bfloat16, kind="ExternalInput")
    a2_d = nc.dram_tensor("a2", [128, NCHUNK, C], dt.bfloat16, kind="ExternalInput")
    f1_d = nc.dram_tensor("fc1t", [C, MID], dt.float32, kind="ExternalInput")
    f2_d = nc.dram_tensor("fc2t", [MID, C], dt.float32, kind="ExternalInput")
    id_d = nc.dram_tensor("ident", [C, C], dt.float32r, kind="ExternalInput")
    out_d = nc.dram_tensor("out", [C, P], dt.float32, kind="ExternalOutput")

    with TileContext(nc) as tc:
        with tc.tile_pool(name="single", bufs=1) as single, \
             tc.tile_pool(name="dram", bufs=1, space="DRAM") as dpool, \
             tc.tile_pool(name="feat", bufs=6) as featp, \
             tc.tile_pool(name="outs", bufs=4) as outsp, \
             tc.tile_pool(name="psum", bufs=8, space="PSUM") as psum, \
             (tc.For_i(0, niter, 1,
                       hint_engines=(mybir.EngineType.PE,
                                     mybir.EngineType.DVE,
                                     mybir.EngineType.SP,
                                     mybir.EngineType.Activation))
              if niter else contextlib.nullcontext()):

            NSPLIT = 2
            HALF = P // NSPLIT
            hsls = [slice(i * HALF, (i + 1) * HALF) for i in range(NSPLIT)]
            # Two HWDGE queues: nc.sync (SP) carries the bulk variant loads,
            # nc.scalar (Activation) carries latency-critical small DMAs.
            # ---- load x (fp32, resident) + weights ----
            xf = single.tile([C, P], dt.float32r)
            for hsl in hsls:
                nc.scalar.dma_start(out=xf[:, hsl], in_=x_d.ap()[:, hsl])
            a1s = single.tile([128, NCHUNK, C], dt.bfloat16)
            nc.scalar.dma_start(out=a1s, in_=a1_d.ap())
            a2s = single.tile([128, NCHUNK, C], dt.bfloat16)
            nc.scalar.dma_start(out=a2s, in_=a2_d.ap())
            f1s = single.tile([C, MID], dt.float32)
            nc.scalar.dma_start(out=f1s, in_=f1_d.ap())
            f2s = single.tile([MID, C], dt.float32)
            nc.scalar.dma_start(out=f2s, in_=f2_d.ap())
            ids = single.tile([C, C], dt.float32r)
            nc.scalar.dma_start(out=ids, in_=id_d.ap())

            # ---- prestage: cast x -> bf16 (+ per-half channel sums),
            # double rows in DRAM.  The whole chain streams in column halves
            # so the first chunk's feature product starts early.
            xb = single.tile([C, P], dt.bfloat16)
            sums_h = [single.tile([C, 1], dt.float32, name=f"sums{h}")
                      for h in range(NSPLIT)]
            xb2_dram = dpool.tile([128, P], dt.bfloat16)
            a_tiles = [single.tile([128, P], dt.bfloat16, name=f"av{k}")
                       for k in range(NA)]
            b_tiles = [single.tile([128, P], dt.bfloat16, name=f"bv{l}")
                       for l in range(NB)]

            for h, hsl in enumerate(hsls):
                nc.scalar.activation(xb[:, hsl], xf[:, hsl],
                                     mybir.ActivationFunctionType.Copy,
                                     accum_out=sums_h[h])
                nc.sync.dma_start(out=xb2_dram[0:C, hsl], in_=xb[:, hsl])
                nc.sync.dma_start(out=xb2_dram[C:128, hsl], in_=xb[:, hsl])

                def load_half(dst, hrow, r):
                    nc.sync.dma_start(out=dst[C * hrow:C * hrow + C, hsl],
                                      in_=xb2_dram[r:r + C, hsl])

                # A0 = [x; x]: one contiguous 128-row load per column half
                nc.sync.dma_start(out=a_tiles[0][:, hsl],
                                  in_=xb2_dram[:, hsl])
                for l in range(NB):
                    load_half(b_tiles[l], 0, 2 * l)
                    load_half(b_tiles[l], 1, 2 * l + 1)
                for k in range(1, NA):
                    load_half(a_tiles[k], 0, 64 - 6 * k)
                    load_half(a_tiles[k], 1, 64 - 6 * k)

            # ---- SE path: s = sigmoid(fc2t.T @ relu(fc1t.T @ sums)) ----
            # the two half-sums accumulate in PSUM across two matmuls
            ps1 = psum.tile([MID, 1], dt.float32, tag="acc")
            for h in range(NSPLIT):
                nc.tensor.matmul(ps1, f1s, sums_h[h], start=(h == 0),
                                 stop=(h == NSPLIT - 1))
            y1 = single.tile([MID, 1], dt.float32)
            nc.scalar.activation(y1, ps1, mybir.ActivationFunctionType.Relu)
            ps2 = psum.tile([C, 1], dt.float32, tag="acc")
            nc.tensor.matmul(ps2, f2s, y1, start=True, stop=True)
            svec = single.tile([C, 1], dt.float32)
            nc.scalar.activation(svec, ps2, mybir.ActivationFunctionType.Sigmoid)

            # s -> DRAM twice (s_int = [s; s]) for the gather DMAs
            s_int = dpool.tile([2 * C], dt.float32)
            nc.scalar.dma_start(out=s_int[0:C][:, None], in_=svec)
            nc.scalar.dma_start(out=s_int[C:2 * C][:, None], in_=svec)

            # gathers: S1b[q, l] = s[j(l, q)] = s_int[qhi + qlo + 2l]  (3 cols)
            s1b = single.tile([128, NB], dt.float32)
            for qhi in range(2):
                nc.scalar.dma_start(
                    out=s1b[64 * qhi:64 * qhi + 64, :],
                    in_=bass.AP(tensor=s_int.tensor,
                                offset=s_int.offset + qhi,
                                ap=[[1, 64], [2, NB]]))
            # S2b[q, k] = s[i(k, q)] = s_int[qlo + 64 - 6k]  (6 cols)
            s2b = single.tile([128, NA], dt.float32)
            for k in range(NA):
                nc.scalar.dma_start(
                    out=s2b[:, k:k + 1],
                    in_=bass.AP(tensor=s_int.tensor,
                                offset=s_int.offset + (64 - 6 * k) % 64,
                                ap=[[0, 2], [1, 64], [0, 1]]))

            # ---- fold s into weights: wc = a1*S1 + a2*S2 (bf16) ----
            # S1 col l serves chunks m = l (mod 3) (strided AP); S2 col k
            # serves chunks 3k..3k+2 (contiguous).  One fused DVE add.
            wc = single.tile([128, NCHUNK, C], dt.bfloat16)
            t1 = single.tile([128, NCHUNK, C], dt.float32)
            t2 = single.tile([128, NCHUNK, C], dt.float32)
            for l in range(NB):
                nc.scalar.mul(t1[:, l::3, :], a1s[:, l::3, :], s1b[:, l:l + 1])
            for k in range(NA):
                ms = slice(3 * k, min(3 * k + 3, NCHUNK))
                nc.scalar.mul(t2[:, ms, :], a2s[:, ms, :], s2b[:, k:k + 1])
            nc.vector.tensor_add(
                wc.rearrange("p a b -> p (a b)"),
                t1.rearrange("p a b -> p (a b)"),
                t2.rearrange("p a b -> p (a b)"))

            # ---- main sweep: column half h finishes (TT -> GEMM -> copy
            # out) while the other half's loads/TTs stream ----
            NSH = NS // NSPLIT
            for h, hsl in enumerate(hsls):
                banks = [psum.tile([C, NSUB], dt.float32, tag="acc",
                                   name=f"bank{h}_{j}") for j in range(NSH)]
                for m in range(NCHUNK):
                    k, l = divmod(m, 3)
                    f = featp.tile([128, HALF], dt.bfloat16, tag="f")
                    nc.vector.tensor_mul(f, a_tiles[k][:, hsl],
                                         b_tiles[l][:, hsl])
                    for j in range(NSH):
                        nc.tensor.matmul(banks[j], wc[:, m, :],
                                         f[:, j * NSUB:(j + 1) * NSUB],
                                         start=(m == 0),
                                         stop=(m == NCHUNK - 1))
                    if m == 5:
                        # conv_b term: += B @ x (float32r, full rate); placed
                        # mid-stream so it runs at warm PE clock.
                        for j in range(NSH):
                            col = h * HALF + j * NSUB
                            nc.tensor.matmul(banks[j], ids,
                                             xf[:, col:col + NSUB],
                                             start=False, stop=False)
                for j in range(NSH):
                    col = h * HALF + j * NSUB
                    ot = outsp.tile([C, NSUB], dt.float32, tag="o")
                    nc.scalar.copy(ot, banks[j])
                    nc.sync.dma_start(out=out_d.ap()[:, col:col + NSUB],
                                      in_=ot)

    nc.compile()
    return nc


def _get_program(niter=None):
    key = ("nc", niter)
    if key not in _CACHE:
        _CACHE[key] = _build_program(niter)
    return _CACHE[key]


def kernel(x, fc1_w, fc2_w, conv_w, conv_b):
    from concourse.bass_utils import run_bass_kernel_spmd

    x = np.asarray(x, np.float32)
    a1, a2, fc1t, fc2t = _host_weights(
        np.asarray(conv_w, np.float32), np.asarray(fc1_w, np.float32),
        np.asarray(fc2_w, np.float32))
    # conv_b contributes sum_i B[c,i]*x_i with B = conv_b.reshape(C, C); the
    # "residual" matmul realizes it with lhsT = B.T (identity-init -> +x).
    ident = np.ascontiguousarray(
        np.asarray(conv_b, np.float32).reshape(C, C).T)
    nc = _get_program()
    in_maps = []
    for b in range(N_CORES):
        in_maps.append({
            "x": np.ascontiguousarray(x[b].reshape(C, P)),
            "a1": a1, "a2": a2, "fc1t": fc1t, "fc2t": fc2t, "ident": ident,
        })
    res = run_bass_kernel_spmd(nc, in_maps, core_ids=list(range(N_CORES)))
    out = np.stack([res.results[b]["out"].reshape(C, H, W)
                    for b in range(N_CORES)], axis=0)
    return out.astype(np.float32)

